# revision 4
# baseline (speedup 1.0000x reference)
"""MARN (multi-attention recurrent network) Trainium2 kernel.

Strategy: data-parallel over batch (B=512 -> 8 cores x 64). Everything on
device is feature-major ([feature -> partitions, (mod, batch) -> free]), so
every matmul has a [128,128] bf16 stationary weight chunk and the per-step
batch tile streaming; all biases are folded in via tiny K<=8 "bias matmuls"
that initialize the PSUM accumulation groups; sigmoid is computed from tanh
(only ACT table set used is exp_and_others: tanh/exp/identity), and the
recurrent z-state feeds the next step through precombined V' = D2m @ Vw so
the z output itself is off the critical chain.
"""

import sys
import numpy as np

for p in ("/opt/trn_rl_repo",):
    if p not in sys.path:
        sys.path.append(p)

import ml_dtypes  # noqa: E402

import concourse.bass as bass  # noqa: E402
import concourse.tile as tile  # noqa: E402
from concourse import bacc, mybir  # noqa: E402
from concourse.bass_utils import run_bass_kernel_spmd  # noqa: E402

T, B, C = 256, 512, 128
NA = 4
NCORES = 8
BL = B // NCORES          # 64 batch per core
W2 = 2 * BL               # 128 = both modalities side by side
BF16 = mybir.dt.bfloat16
F32 = mybir.dt.float32
AF = mybir.ActivationFunctionType

PERM = [0, 1, 3, 2]       # gate chunk order in psum: f, i, ch, o
SCALE = [0.5, 0.5, 1.0, 0.5]
PREFETCH = 6

_cache = {}


def _ps_cols(W):
    """Permute+scale the last (4C) dim into [f,i,ch,o] chunk order."""
    chunks = [W[..., p * C:(p + 1) * C] * s for p, s in zip(PERM, SCALE)]
    return np.concatenate(chunks, axis=-1)


def _bf(x):
    return np.ascontiguousarray(np.asarray(x, np.float32)).astype(ml_dtypes.bfloat16)


def _prep_weights(inp):
    Ww, Wb = np.asarray(inp['Ww'], np.float32), np.asarray(inp['Wb'], np.float32)
    Uw, Ub = np.asarray(inp['Uw'], np.float32), np.asarray(inp['Ub'], np.float32)
    Vw, Vb = np.asarray(inp['Vw'], np.float32), np.asarray(inp['Vb'], np.float32)
    A1, a1 = np.asarray(inp['A1'], np.float32), np.asarray(inp['a1'], np.float32)
    A2, a2 = np.asarray(inp['A2'], np.float32), np.asarray(inp['a2'], np.float32)
    D10, e10 = np.asarray(inp['D10'], np.float32), np.asarray(inp['e10'], np.float32)
    D20, e20 = np.asarray(inp['D20'], np.float32), np.asarray(inp['e20'], np.float32)
    D11, e11 = np.asarray(inp['D11'], np.float32), np.asarray(inp['e11'], np.float32)
    D21, e21 = np.asarray(inp['D21'], np.float32), np.asarray(inp['e21'], np.float32)

    bias0 = _ps_cols(Wb + Ub + Vb + e20 @ Vw)   # [512] per-mod combined bias
    bias1 = _ps_cols(Wb + Ub + Vb + e21 @ Vw)
    biasW = _ps_cols(Wb)                        # t=0: W-bias only
    bg = np.zeros((8, C), np.float32)
    bg0 = np.zeros((8, C), np.float32)
    for j in range(4):
        for m in range(2):
            src = bias0 if m == 0 else bias1
            bg[2 * j + m] = src[j * C:(j + 1) * C]
            bg0[2 * j + m] = biasW[j * C:(j + 1) * C]
    ba2 = a2.reshape(8, C)
    ind = np.zeros((8, 8 * BL), np.float32)
    for k in range(8):
        ind[k, k * BL:(k + 1) * BL] = 1.0

    return {
        'wW': _bf(_ps_cols(Ww)),
        'wU': _bf(_ps_cols(Uw)),
        'wV0': _bf(_ps_cols(D20 @ Vw)),
        'wV1': _bf(_ps_cols(D21 @ Vw)),
        'wA1': _bf(np.stack([A1[0:C], A1[C:2 * C]], axis=1)),        # [128,2,128]
        'wA2': _bf(A2),                                              # [128,1024]
        'wD10': _bf(np.stack([D10[k * C:(k + 1) * C] for k in range(4)], axis=1)),
        'wD11': _bf(np.stack([D11[k * C:(k + 1) * C] for k in range(4)], axis=1)),
        'wD20': _bf(D20),
        'wD21': _bf(D21),
        'bg': _bf(bg),
        'bg0': _bf(bg0),
        'ba2': _bf(ba2),
        'bu': _bf(np.stack([e10, e11])),
        'bz': _bf(np.stack([e20, e21])),
        'ind': _bf(ind),
        'ba1': np.ascontiguousarray(a1[:, None], dtype=np.float32),  # [128,1]
    }


def _free_ap(t, free_dims, offset_elems=0):
    """AP over SBUF tile `t` with custom free dims [[step,count],...]."""
    base = t[:, :]
    return bass.AP(tensor=base.tensor, offset=base.offset + offset_elems,
                   ap=[list(base.ap[0])] + [list(d) for d in free_dims])


def _build_program(nsteps=T):
    nc = bacc.Bacc("TRN2", target_bir_lowering=False, debug=False)

    x_d = nc.dram_tensor("x", [nsteps, C, W2], BF16, kind="ExternalInput")
    out_d = nc.dram_tensor("out", [nsteps, C, W2], F32, kind="ExternalOutput")
    wd = {}
    for name, shape in [
        ('wW', [C, 512]), ('wU', [C, 512]), ('wV0', [C, 512]), ('wV1', [C, 512]),
        ('wA1', [C, 2, C]), ('wA2', [C, 1024]),
        ('wD10', [C, 4, C]), ('wD11', [C, 4, C]),
        ('wD20', [C, C]), ('wD21', [C, C]),
        ('bg', [8, C]), ('bg0', [8, C]), ('ba2', [8, C]),
        ('bu', [2, C]), ('bz', [2, C]), ('ind', [8, 8 * BL]),
    ]:
        wd[name] = nc.dram_tensor(name, shape, BF16, kind="ExternalInput")
    wd['ba1'] = nc.dram_tensor('ba1', [C, 1], F32, kind="ExternalInput")

    with tile.TileContext(nc) as tc:
        with (
            tc.tile_pool(name="wpool", bufs=1) as wpool,
            tc.tile_pool(name="xpool", bufs=PREFETCH) as xpool,
            tc.tile_pool(name="tmp", bufs=2) as tmp,
            tc.tile_pool(name="zpool", bufs=3) as zpool,
            tc.tile_pool(name="gpsum", bufs=2, space="PSUM") as gpsum,
            tc.tile_pool(name="lpsum", bufs=2, space="PSUM") as lpsum,
            tc.tile_pool(name="t1psum", bufs=1, space="PSUM") as t1psum,
            tc.tile_pool(name="upsum", bufs=1, space="PSUM") as upsum,
            tc.tile_pool(name="zpsum", bufs=1, space="PSUM") as zpsum,
        ):
            # ---- load weights (once) ----
            w = {}
            for name, t_d in wd.items():
                shape = list(t_d.shape)
                dt = BF16 if name != 'ba1' else F32
                w[name] = wpool.tile(shape, dt, tag=name, name=name)
                nc.sync.dma_start(out=w[name][:], in_=t_d[:])
            dacc = wpool.tile([C, 1], F32, tag="dacc", name="dacc")  # dummy accum target

            x_tiles = {}
            for t in range(min(PREFETCH, nsteps)):
                xt = xpool.tile([C, W2], BF16, tag="x", name="xt")
                nc.sync.dma_start(out=xt[:], in_=x_d[t])
                x_tiles[t] = xt

            # ---- t=0 gates: bias(W only) + W-matmuls ----
            g_cur = gpsum.tile([C, 512], F32, tag="g")
            nc.tensor.matmul(g_cur[:], w['bg0'][:], w['ind'][:],
                             start=True, stop=False, skip_group_check=True)
            for j in range(4):
                nc.tensor.matmul(g_cur[:, j * C:(j + 1) * C],
                                 w['wW'][:, j * C:(j + 1) * C], x_tiles[0][:],
                                 start=False, stop=(j == 3), skip_group_check=True)

            c_prev = None
            for t in range(nsteps):
                # ---- gates -> T -> c, h ----
                Tt = tmp.tile([C, 512], F32, tag="T")
                nc.scalar.activation(out=Tt[:], in_=g_cur[:], func=AF.Tanh)
                c_new = tmp.tile([C, W2], F32, tag="c")
                if c_prev is None:
                    nc.vector.affine_mul_reduce(
                        out=c_new[:], accum_out=dacc[:], in0=Tt[:, 128:256],
                        in1=Tt[:, 256:384], scale=0.5, bias=0.5)
                else:
                    m1 = tmp.tile([C, W2], F32, tag="m1")
                    m2 = tmp.tile([C, W2], F32, tag="m2")
                    nc.vector.affine_mul_reduce(
                        out=m1[:], accum_out=dacc[:], in0=Tt[:, 0:128],
                        in1=c_prev[:], scale=0.5, bias=0.5)
                    nc.vector.affine_mul_reduce(
                        out=m2[:], accum_out=dacc[:], in0=Tt[:, 128:256],
                        in1=Tt[:, 256:384], scale=0.5, bias=0.5)
                    nc.vector.tensor_add(c_new[:], m1[:], m2[:])
                c_prev = c_new
                tc_t = tmp.tile([C, W2], F32, tag="tc")
                nc.scalar.activation(out=tc_t[:], in_=c_new[:], func=AF.Tanh)
                h = tmp.tile([C, W2], BF16, tag="h")
                nc.vector.affine_mul_reduce(
                    out=h[:], accum_out=dacc[:], in0=Tt[:, 384:512],
                    in1=tc_t[:], scale=0.5, bias=0.5)

                # ---- next-step gates: bias + W(x_{t+1}) + U(h_t) ----
                g_next = None
                if t + 1 < nsteps:
                    g_next = gpsum.tile([C, 512], F32, tag="g")
                    nc.tensor.matmul(g_next[:], w['bg'][:], w['ind'][:],
                                     start=True, stop=False, skip_group_check=True)
                    for j in range(4):
                        nc.tensor.matmul(g_next[:, j * C:(j + 1) * C],
                                         w['wW'][:, j * C:(j + 1) * C],
                                         x_tiles[t + 1][:],
                                         start=False, stop=False,
                                         skip_group_check=True)
                    for j in range(4):
                        nc.tensor.matmul(g_next[:, j * C:(j + 1) * C],
                                         w['wU'][:, j * C:(j + 1) * C], h[:],
                                         start=False, stop=False,
                                         skip_group_check=True)

                # ---- attention MLP ----
                t1p = t1psum.tile([C, BL], F32, tag="t1p")
                nc.tensor.matmul(t1p[:], w['wA1'][:, 0, :], h[:, 0:BL],
                                 start=True, stop=False)
                nc.tensor.matmul(t1p[:], w['wA1'][:, 1, :], h[:, BL:W2],
                                 start=False, stop=True)
                t1 = tmp.tile([C, BL], BF16, tag="t1")
                nc.scalar.activation(out=t1[:], in_=t1p[:], func=AF.Tanh,
                                     bias=w['ba1'][:])
                lp = lpsum.tile([C, 512], F32, tag="lp")
                nc.tensor.matmul(lp[:], w['ba2'][:], w['ind'][:],
                                 start=True, stop=False, skip_group_check=True)
                for k in range(8):
                    nc.tensor.matmul(lp[:, k * BL:(k + 1) * BL],
                                     w['wA2'][:, k * C:(k + 1) * C], t1[:],
                                     start=False, stop=(k == 7),
                                     skip_group_check=True)
                e = tmp.tile([C, 512], F32, tag="e")
                nc.scalar.activation(out=e[:], in_=lp[:], func=AF.Exp)

                # ---- softmax over the 4 attention heads ----
                s1 = tmp.tile([C, 256], F32, tag="s1")
                nc.vector.tensor_add(s1[:], e[:, 0:256], e[:, 256:512])
                s = tmp.tile([C, 128], F32, tag="s")
                nc.vector.tensor_add(s[:], s1[:, 0:128], s1[:, 128:256])
                r = tmp.tile([C, 128], F32, tag="r")
                nc.vector.reciprocal_approx_fast(out=r[:], in_=s[:])
                # G[p, (half*2+par)*64+b] = r[p, par*64+b] * h[p, half*64+b]
                G = tmp.tile([C, 256], F32, tag="G")
                r_b = _free_ap(r, [[0, 2], [BL, 2], [1, BL]])
                h_b = _free_ap(h, [[BL, 2], [0, 2], [1, BL]])
                nc.vector.tensor_mul(_free_ap(G, [[2 * BL, 2], [BL, 2], [1, BL]]),
                                     r_b, h_b)
                att = tmp.tile([C, 512], BF16, tag="att")
                for half in range(2):
                    e_v = _free_ap(e, [[2 * BL, 2], [BL, 2], [1, BL]],
                                   offset_elems=half * 4 * BL)
                    g_v = _free_ap(G, [[0, 2], [BL, 2], [1, BL]],
                                   offset_elems=half * 2 * BL)
                    a_v = _free_ap(att, [[2 * BL, 2], [BL, 2], [1, BL]],
                                   offset_elems=half * 4 * BL)
                    nc.vector.tensor_mul(a_v, e_v, g_v)

                # ---- dim-reduce nets ----
                up = upsum.tile([C, W2], F32, tag="up")
                nc.tensor.matmul(up[:], w['bu'][:], w['ind'][0:2, 0:W2],
                                 start=True, stop=False, skip_group_check=True)
                for k in range(4):
                    nc.tensor.matmul(up[:, 0:BL], w['wD10'][:, k, :],
                                     att[:, k * BL:(k + 1) * BL],
                                     start=False, stop=False,
                                     skip_group_check=True)
                for k in range(4):
                    nc.tensor.matmul(up[:, BL:W2], w['wD11'][:, k, :],
                                     att[:, (4 + k) * BL:(5 + k) * BL],
                                     start=False, stop=(k == 3),
                                     skip_group_check=True)
                u = tmp.tile([C, W2], BF16, tag="u")
                nc.scalar.activation(out=u[:], in_=up[:], func=AF.Tanh)

                # ---- V' into next gates (z-state shortcut) ----
                if g_next is not None:
                    for j in range(4):
                        nc.tensor.matmul(g_next[:, j * C:j * C + BL],
                                         w['wV0'][:, j * C:(j + 1) * C],
                                         u[:, 0:BL],
                                         start=False, stop=False,
                                         skip_group_check=True)
                    for j in range(4):
                        nc.tensor.matmul(g_next[:, j * C + BL:(j + 1) * C],
                                         w['wV1'][:, j * C:(j + 1) * C],
                                         u[:, BL:W2],
                                         start=False, stop=(j == 3),
                                         skip_group_check=True)

                # ---- z output ----
                zp = zpsum.tile([C, W2], F32, tag="zp")
                nc.tensor.matmul(zp[:], w['bz'][:], w['ind'][0:2, 0:W2],
                                 start=True, stop=False, skip_group_check=True)
                nc.tensor.matmul(zp[:, 0:BL], w['wD20'][:], u[:, 0:BL],
                                 start=False, stop=False, skip_group_check=True)
                nc.tensor.matmul(zp[:, BL:W2], w['wD21'][:], u[:, BL:W2],
                                 start=False, stop=True, skip_group_check=True)
                z_out = zpool.tile([C, W2], F32, tag="z")
                nc.scalar.activation(out=z_out[:], in_=zp[:], func=AF.Identity)
                nc.sync.dma_start(out=out_d[t], in_=z_out[:])

                # ---- x prefetch ----
                if t + PREFETCH < nsteps:
                    xt = xpool.tile([C, W2], BF16, tag="x", name="xt")
                    nc.sync.dma_start(out=xt[:], in_=x_d[t + PREFETCH])
                    x_tiles[t + PREFETCH] = xt
                g_cur = g_next

    nc.compile()
    return nc


def kernel(**inputs):
    eeg = np.asarray(inputs['eeg'], np.float32)
    eog = np.asarray(inputs['eog'], np.float32)
    wmap = _prep_weights(inputs)

    if 'nc' not in _cache:
        _cache['nc'] = _build_program(T)
    nc = _cache['nc']

    in_maps = []
    for i in range(NCORES):
        sl = slice(i * BL, (i + 1) * BL)
        xcat = np.concatenate([eeg[:, sl, :].transpose(0, 2, 1),
                               eog[:, sl, :].transpose(0, 2, 1)], axis=2)
        m = dict(wmap)
        m['x'] = np.ascontiguousarray(xcat).astype(ml_dtypes.bfloat16)
        in_maps.append(m)

    res = run_bass_kernel_spmd(nc, in_maps, list(range(NCORES)))
    full = np.empty((T, B, 2 * C), np.float32)
    for i in range(NCORES):
        arr = np.asarray(res.results[i]['out'])  # [T, 128, 128]
        full[:, i * BL:(i + 1) * BL, :] = (
            arr.reshape(T, C, 2, BL).transpose(0, 3, 2, 1).reshape(T, BL, 2 * C))
    return full
